# revision 35
# baseline (speedup 1.0000x reference)
"""GRNN over perfect binary trees (jet embeddings) on 8 Trainium2 cores.

Strategy
--------
The model is a bottom-up pass over 64 independent depth-12 perfect binary
trees: per level j,  u = tanh(c_j @ w_u.T + b_u)  and for inner nodes
emb_j = tanh([h_L, h_R, u] @ w_h.T + b_h)  with h_L/h_R gathered from
level j+1 by per-level child indices.

Host-side we relabel nodes by walking the trees down from the roots:
order_0 = [0..63],  order_{j+1} = [children_j[order_j][:,0], children_j[order_j][:,1]]
(left children first, then right children).  In the relabeled arrays the
children of the node at position p (of S) are at positions p and S+p of
the next level, for ANY children input (the walk duplicates/clamps
exactly like the reference's clipped gather).  All gathers become
contiguous block reads, so the device kernel is a pure matmul+tanh
stream with no indirect addressing: every level's embedding stays in
SBUF.

Sharding: core d owns roots 8d..8d+8 (a contiguous slice of every
relabeled level) -> 8 fully independent per-core problems, no collectives.

The kernel is ScalarE-bound: every node needs a 128-wide tanh(u) and
every inner node a 128-wide tanh(emb); ScalarE evaluates tanh at 1
elem/cycle/lane with no dtype acceleration, ~92us/core of mandatory
activation work.  The pipeline therefore keeps ScalarE busy 100% of the
time and hides all matmul/DMA work behind it:
  - PSUM is split into a 4-bank tile pa and two 2-bank tiles pb0/pb1.
    Leaf chunks alternate between pa and the pb pair (u-matmul fills one
    while tanh drains the other).  Dependency tracking is whole-tile, so
    anything that must overlap needs separate tiles.
  - Inner chunks put the u-stream in pa and the h-stream in pb0/pb1
    (matmuls filling pb1 overlap the activation draining pb0).
  - The u-stage (DMA + K=7 matmul + tanh) runs one chunk ahead of the
    h-stage so the W_hu operand is always ready.
Everything is feature-major [128=H, cols]; fp16 operands, fp32 PSUM.
"""

import numpy as np
from contextlib import ExitStack

import concourse.bass as bass
import concourse.bacc as bacc
import concourse.tile as tile
from concourse import mybir
from concourse.bass_utils import run_bass_kernel_spmd

# ---- static problem geometry (hardcoded per contest rules) ----
B = 64
DEPTH = 12
N_FEAT = 7
N_HID = 128
N_CORES = 8
RPC = B // N_CORES  # roots per core

LEVEL_SIZES = [B * (1 << j) for j in range(DEPTH + 1)]
OFFSETS = np.concatenate([[0], np.cumsum(LEVEL_SIZES)]).astype(np.int64)
INNER_OFF = np.concatenate([[0], np.cumsum(LEVEL_SIZES[:-1])]).astype(np.int64)

# per-core level sizes and their column offsets in the compute-order
# (leaf level first) contents buffer
PC_SIZES = {j: RPC << j for j in range(DEPTH + 1)}
PC_TOTAL = sum(PC_SIZES.values())  # 65528
PC_OFF = {}
_acc = 0
for _j in range(DEPTH, -1, -1):
    PC_OFF[_j] = _acc
    _acc += PC_SIZES[_j]

CHUNK = 2048
HALF = 1024
F16 = mybir.dt.float16
F32 = mybir.dt.float32

_COMPILED = {}  # cache: built+compiled Bass program


def _build_program():
    nc = bacc.Bacc("TRN2", target_bir_lowering=False, debug=False,
                   num_devices=N_CORES)

    c_d = nc.dram_tensor("c", [N_FEAT, PC_TOTAL], F16, kind="ExternalInput").ap()
    wu_d = nc.dram_tensor("wu", [N_FEAT, N_HID], F16, kind="ExternalInput").ap()
    whl_d = nc.dram_tensor("whl", [N_HID, N_HID], F16, kind="ExternalInput").ap()
    whr_d = nc.dram_tensor("whr", [N_HID, N_HID], F16, kind="ExternalInput").ap()
    whu_d = nc.dram_tensor("whu", [N_HID, N_HID], F16, kind="ExternalInput").ap()
    bu_d = nc.dram_tensor("bu", [N_HID, 1], F32, kind="ExternalInput").ap()
    bh_d = nc.dram_tensor("bh", [N_HID, 1], F32, kind="ExternalInput").ap()
    out_d = nc.dram_tensor("out", [N_HID, RPC], F32, kind="ExternalOutput").ap()

    with tile.TileContext(nc) as tc:
        with ExitStack() as ctx:
            _kernel_body(ctx, tc, c_d, wu_d, whl_d, whr_d, whu_d, bu_d, bh_d,
                         out_d)

    nc.compile()
    return nc


def _kernel_body(ctx, tc, c_d, wu_d, whl_d, whr_d, whu_d, bu_d, bh_d, out_d):
    nc = tc.nc
    TANH = mybir.ActivationFunctionType.Tanh

    wpool = ctx.enter_context(tc.tile_pool(name="weights", bufs=1))
    epool = ctx.enter_context(tc.tile_pool(name="emb", bufs=1))
    cpool = ctx.enter_context(tc.tile_pool(name="cstage", bufs=5))
    upool = ctx.enter_context(tc.tile_pool(name="ustage", bufs=4))
    opool = ctx.enter_context(tc.tile_pool(name="outbuf", bufs=1))
    papool = ctx.enter_context(tc.tile_pool(name="pa", bufs=1, space="PSUM"))
    pbpool = ctx.enter_context(tc.tile_pool(name="pb", bufs=1, space="PSUM"))

    wu_sb = wpool.tile([N_FEAT, N_HID], F16)
    whl_sb = wpool.tile([N_HID, N_HID], F16)
    whr_sb = wpool.tile([N_HID, N_HID], F16)
    whu_sb = wpool.tile([N_HID, N_HID], F16)
    bu_sb = wpool.tile([N_HID, 1], F32)
    bh_sb = wpool.tile([N_HID, 1], F32)

    pa = papool.tile([N_HID, CHUNK], F32, name="pa")
    pb = [pbpool.tile([N_HID, HALF], F32, name=f"pb{k}") for k in range(2)]

    # weights go via the gpsimd DMA queue so the sync queue can issue the
    # first contents chunks immediately (the queues run in parallel and
    # each DMA costs ~0.9us of issue time on its queue)
    nc.gpsimd.dma_start(wu_sb[:], wu_d)
    nc.gpsimd.dma_start(bu_sb[:], bu_d)
    nc.gpsimd.dma_start(whl_sb[:], whl_d)
    nc.gpsimd.dma_start(whr_sb[:], whr_d)
    nc.gpsimd.dma_start(whu_sb[:], whu_d)
    nc.gpsimd.dma_start(bh_sb[:], bh_d)

    # embedding arenas, ping-pong between consecutive levels
    e_tiles = {}
    for j in range(DEPTH, 0, -1):
        tag = "ping" if j % 2 == 0 else "pong"
        e_tiles[j] = epool.tile([N_HID, PC_SIZES[j]], F16, tag=tag,
                                name=f"e{j}")

    # Station list, leaf level first.  Levels j <= TAIL_J are a pure
    # latency chain (sizes <= 1024), so their u-activations are hoisted
    # into one combined "uall" chunk: that removes the u-matmul+tanh from
    # every tail level's critical path.
    TAIL_J = 7
    TAIL_TOT = sum(PC_SIZES[j] for j in range(TAIL_J + 1))  # 2040
    stations = []
    for j in range(DEPTH, TAIL_J, -1):
        S = PC_SIZES[j]
        for a in range(0, S, CHUNK):
            stations.append(("leaf" if j == DEPTH else "inner", j, a,
                             min(CHUNK, S - a)))
    stations.append(("uall", TAIL_J, 0, TAIL_TOT))
    for j in range(TAIL_J, -1, -1):
        stations.append(("tailh", j, 0, PC_SIZES[j]))

    u_all = wpool.tile([N_HID, TAIL_TOT], F16, name="uall")
    state = {"leaf_ct": 0, "next_u": 0, "inner_ct": 0}
    u_tiles = {}

    def emit_u(i):
        kind, j, a, n = stations[i]
        col0 = PC_OFF[j] + a
        cst = cpool.tile([N_FEAT, CHUNK], F16, tag="cst", name=f"cst{i}")
        nc.sync.dma_start(cst[:, :n], c_d[:, col0:col0 + n])
        if kind == "leaf" and (state["leaf_ct"] % 2 == 1
                               or state["leaf_ct"] >= 14):
            # odd leaf chunks go to the pb pair: two 1024-col halves.
            # The last two leaf chunks both use the pair so pa frees two
            # periods early for the first inner chunks' u-stage (kills
            # the leaf->inner pipeline bubble).
            state["leaf_ct"] += 1
            for k in range(2):
                h0 = k * HALF
                hn = min(HALF, max(0, n - h0))
                if hn == 0:
                    continue
                for s in range(0, hn, 512):
                    w = min(512, hn - s)
                    nc.tensor.matmul(pb[k][:, s:s + w], wu_sb[:],
                                     cst[:, h0 + s:h0 + s + w],
                                     start=True, stop=True)
                nc.scalar.activation(e_tiles[j][:, a + h0:a + h0 + hn],
                                     pb[k][:, :hn], TANH,
                                     bias=bu_sb[:, 0:1])
            return
        if kind == "leaf":
            state["leaf_ct"] += 1
        if kind == "inner" and state.get("first_inner", True):
            # at the leaf->inner boundary pa is still hot with the last
            # even leaf chunk; the pb pair is free one period earlier, so
            # the first inner u goes there to avoid a pipeline bubble
            state["first_inner"] = False
            u_sb = upool.tile([N_HID, CHUNK], F16, tag="u", name=f"u{i}")
            for k in range(2):
                h0 = k * HALF
                hn = min(HALF, max(0, n - h0))
                if hn == 0:
                    continue
                for s in range(0, hn, 512):
                    w = min(512, hn - s)
                    nc.tensor.matmul(pb[k][:, s:s + w], wu_sb[:],
                                     cst[:, h0 + s:h0 + s + w],
                                     start=True, stop=True)
                nc.scalar.activation(u_sb[:, h0:h0 + hn], pb[k][:, :hn],
                                     TANH, bias=bu_sb[:, 0:1])
            u_tiles[i] = u_sb
            return
        for s in range(0, n, 512):
            w = min(512, n - s)
            nc.tensor.matmul(pa[:, s:s + w], wu_sb[:], cst[:, s:s + w],
                             start=True, stop=True)
        if kind == "leaf":
            nc.scalar.activation(e_tiles[j][:, a:a + n], pa[:, :n], TANH,
                                 bias=bu_sb[:, 0:1])
        elif kind == "uall":
            nc.scalar.activation(u_all[:, :n], pa[:, :n], TANH,
                                 bias=bu_sb[:, 0:1])
        else:
            u_sb = upool.tile([N_HID, CHUNK], F16, tag="u", name=f"u{i}")
            nc.scalar.activation(u_sb[:, :n], pa[:, :n], TANH,
                                 bias=bu_sb[:, 0:1])
            u_tiles[i] = u_sb

    def pump_u(upto):
        # keep the u-stream two stations ahead of the h-stream
        while state["next_u"] < len(stations) and state["next_u"] <= upto:
            if stations[state["next_u"]][0] != "tailh":
                emit_u(state["next_u"])
            state["next_u"] += 1

    for i, (kind, j, a, n) in enumerate(stations):
        pump_u(i + 2)
        if kind in ("leaf", "uall"):
            continue
        S = PC_SIZES[j]
        eprev = e_tiles[j + 1]
        if kind == "inner":
            u_sb = u_tiles.pop(i)
        # process the h-stream in 1024-col halves, double-buffered across
        # pb0/pb1: matmuls filling one overlap the activation of the other
        for k, h0 in enumerate(range(0, n, HALF)):
            hn = min(HALF, n - h0)
            pt = pb[(k if kind == "inner" else j) % 2]
            # group by stationary weight to amortize LDWEIGHTS
            for s in range(0, hn, 512):
                w = min(512, hn - s)
                nc.tensor.matmul(pt[:, s:s + w], whl_sb[:],
                                 eprev[:, a + h0 + s:a + h0 + s + w],
                                 start=True, stop=False)
            for s in range(0, hn, 512):
                w = min(512, hn - s)
                nc.tensor.matmul(pt[:, s:s + w], whr_sb[:],
                                 eprev[:, S + a + h0 + s:S + a + h0 + s + w],
                                 start=False, stop=False)
            for s in range(0, hn, 512):
                w = min(512, hn - s)
                if kind == "inner":
                    uop = u_sb[:, h0 + s:h0 + s + w]
                else:
                    uo = PC_OFF[j] - PC_OFF[TAIL_J]
                    uop = u_all[:, uo + h0 + s:uo + h0 + s + w]
                nc.tensor.matmul(pt[:, s:s + w], whu_sb[:], uop,
                                 start=False, stop=True)
            if j > 0:
                nc.scalar.activation(e_tiles[j][:, a + h0:a + h0 + hn],
                                     pt[:, :hn], TANH, bias=bh_sb[:, 0:1])
            else:
                out_sb = opool.tile([N_HID, RPC], F32)
                nc.scalar.activation(out_sb[:], pt[:, 0:RPC], TANH,
                                     bias=bh_sb[:, 0:1])
                nc.sync.dma_start(out_d, out_sb[:])
        if kind == "inner":
            state["inner_ct"] += 1


def _preprocess(contents, children):
    """Relabel nodes so children of position p live at p, S+p; return
    per-core feature-major fp16 contents slices (leaf level first)."""
    contents = np.asarray(contents, dtype=np.float32)
    children = np.asarray(children)
    clipped = []
    for j in range(DEPTH):
        ch = children[INNER_OFF[j]:INNER_OFF[j + 1]]
        clipped.append(np.clip(ch, 0, LEVEL_SIZES[j + 1] - 1).astype(np.int64))

    per_core = []
    for d in range(N_CORES):
        # per-core walk: children of relabeled node p (of S) are at
        # positions p (left) and S + p (right) of the next level
        o = np.arange(d * RPC, (d + 1) * RPC, dtype=np.int64)
        segs = [contents[OFFSETS[0] + o]]
        for j in range(DEPTH):
            sel = clipped[j][o]
            o = np.concatenate([sel[:, 0], sel[:, 1]])
            segs.append(contents[OFFSETS[j + 1] + o])
        segs.reverse()  # leaf level first in the device buffer
        Cd = np.concatenate(segs, axis=0)  # [PC_TOTAL, 7]
        per_core.append(np.ascontiguousarray(Cd.T.astype(np.float16)))
    return per_core


def kernel(contents, children, w_u, b_u, w_h, b_h):
    contents = np.asarray(contents)
    children = np.asarray(children)
    w_u = np.asarray(w_u, dtype=np.float32)
    b_u = np.asarray(b_u, dtype=np.float32)
    w_h = np.asarray(w_h, dtype=np.float32)
    b_h = np.asarray(b_h, dtype=np.float32)

    per_core_c = _preprocess(contents, children)

    wu_t = np.ascontiguousarray(w_u.T.astype(np.float16))              # [7,128]
    whl_t = np.ascontiguousarray(w_h[:, 0:128].T.astype(np.float16))   # [128,128]
    whr_t = np.ascontiguousarray(w_h[:, 128:256].T.astype(np.float16))
    whu_t = np.ascontiguousarray(w_h[:, 256:384].T.astype(np.float16))
    bu_c = np.ascontiguousarray(b_u.reshape(N_HID, 1))
    bh_c = np.ascontiguousarray(b_h.reshape(N_HID, 1))

    if "nc" not in _COMPILED:
        _COMPILED["nc"] = _build_program()
    nc = _COMPILED["nc"]

    in_maps = []
    for d in range(N_CORES):
        in_maps.append({
            "c": per_core_c[d],
            "wu": wu_t, "whl": whl_t, "whr": whr_t, "whu": whu_t,
            "bu": bu_c, "bh": bh_c,
        })
    res = run_bass_kernel_spmd(nc, in_maps, list(range(N_CORES)))

    out = np.empty((B, N_HID), dtype=np.float32)
    for d in range(N_CORES):
        out[d * RPC:(d + 1) * RPC, :] = res.results[d]["out"].T
    return out


# revision 36
# speedup vs baseline: 1.0069x; 1.0069x over previous
"""GRNN over perfect binary trees (jet embeddings) on 8 Trainium2 cores.

Strategy
--------
The model is a bottom-up pass over 64 independent depth-12 perfect binary
trees: per level j,  u = tanh(c_j @ w_u.T + b_u)  and for inner nodes
emb_j = tanh([h_L, h_R, u] @ w_h.T + b_h)  with h_L/h_R gathered from
level j+1 by per-level child indices.

Host-side we relabel nodes by walking the trees down from the roots:
order_0 = [0..63],  order_{j+1} = [children_j[order_j][:,0], children_j[order_j][:,1]]
(left children first, then right children).  In the relabeled arrays the
children of the node at position p (of S) are at positions p and S+p of
the next level, for ANY children input (the walk duplicates/clamps
exactly like the reference's clipped gather).  All gathers become
contiguous block reads, so the device kernel is a pure matmul+tanh
stream with no indirect addressing: every level's embedding stays in
SBUF.

Sharding: core d owns roots 8d..8d+8 (a contiguous slice of every
relabeled level) -> 8 fully independent per-core problems, no collectives.

The kernel is ScalarE-bound: every node needs a 128-wide tanh(u) and
every inner node a 128-wide tanh(emb); ScalarE evaluates tanh at 1
elem/cycle/lane with no dtype acceleration, ~92us/core of mandatory
activation work.  The pipeline therefore keeps ScalarE busy 100% of the
time and hides all matmul/DMA work behind it:
  - PSUM is split into a 4-bank tile pa and two 2-bank tiles pb0/pb1.
    Leaf chunks alternate between pa and the pb pair (u-matmul fills one
    while tanh drains the other).  Dependency tracking is whole-tile, so
    anything that must overlap needs separate tiles.
  - Inner chunks put the u-stream in pa and the h-stream in pb0/pb1
    (matmuls filling pb1 overlap the activation draining pb0).
  - The u-stage (DMA + K=7 matmul + tanh) runs one chunk ahead of the
    h-stage so the W_hu operand is always ready.
Everything is feature-major [128=H, cols]; fp16 operands, fp32 PSUM.
"""

import numpy as np
from contextlib import ExitStack

import concourse.bass as bass
import concourse.bacc as bacc
import concourse.tile as tile
from concourse import mybir
from concourse.bass_utils import run_bass_kernel_spmd

# ---- static problem geometry (hardcoded per contest rules) ----
B = 64
DEPTH = 12
N_FEAT = 7
N_HID = 128
N_CORES = 8
RPC = B // N_CORES  # roots per core

LEVEL_SIZES = [B * (1 << j) for j in range(DEPTH + 1)]
OFFSETS = np.concatenate([[0], np.cumsum(LEVEL_SIZES)]).astype(np.int64)
INNER_OFF = np.concatenate([[0], np.cumsum(LEVEL_SIZES[:-1])]).astype(np.int64)

# per-core level sizes and their column offsets in the compute-order
# (leaf level first) contents buffer
PC_SIZES = {j: RPC << j for j in range(DEPTH + 1)}
PC_TOTAL = sum(PC_SIZES.values())  # 65528
PC_OFF = {}
_acc = 0
for _j in range(DEPTH, -1, -1):
    PC_OFF[_j] = _acc
    _acc += PC_SIZES[_j]

CHUNK = 2048
HALF = 1024
F16 = mybir.dt.float16
F32 = mybir.dt.float32

_COMPILED = {}  # cache: built+compiled Bass program


def _build_program():
    nc = bacc.Bacc("TRN2", target_bir_lowering=False, debug=False,
                   num_devices=N_CORES)

    c_d = nc.dram_tensor("c", [N_FEAT, PC_TOTAL], F16, kind="ExternalInput").ap()
    wu_d = nc.dram_tensor("wu", [N_FEAT, N_HID], F16, kind="ExternalInput").ap()
    whl_d = nc.dram_tensor("whl", [N_HID, N_HID], F16, kind="ExternalInput").ap()
    whr_d = nc.dram_tensor("whr", [N_HID, N_HID], F16, kind="ExternalInput").ap()
    whu_d = nc.dram_tensor("whu", [N_HID, N_HID], F16, kind="ExternalInput").ap()
    bu_d = nc.dram_tensor("bu", [N_HID, 1], F32, kind="ExternalInput").ap()
    bh_d = nc.dram_tensor("bh", [N_HID, 1], F32, kind="ExternalInput").ap()
    out_d = nc.dram_tensor("out", [N_HID, RPC], F32, kind="ExternalOutput").ap()

    with tile.TileContext(nc) as tc:
        with ExitStack() as ctx:
            _kernel_body(ctx, tc, c_d, wu_d, whl_d, whr_d, whu_d, bu_d, bh_d,
                         out_d)

    nc.compile()
    return nc


def _kernel_body(ctx, tc, c_d, wu_d, whl_d, whr_d, whu_d, bu_d, bh_d, out_d):
    nc = tc.nc
    TANH = mybir.ActivationFunctionType.Tanh

    wpool = ctx.enter_context(tc.tile_pool(name="weights", bufs=1))
    epool = ctx.enter_context(tc.tile_pool(name="emb", bufs=1))
    cpool = ctx.enter_context(tc.tile_pool(name="cstage", bufs=5))
    upool = ctx.enter_context(tc.tile_pool(name="ustage", bufs=4))
    opool = ctx.enter_context(tc.tile_pool(name="outbuf", bufs=1))
    papool = ctx.enter_context(tc.tile_pool(name="pa", bufs=1, space="PSUM"))
    pbpool = ctx.enter_context(tc.tile_pool(name="pb", bufs=1, space="PSUM"))

    wu_sb = wpool.tile([N_FEAT, N_HID], F16)
    whl_sb = wpool.tile([N_HID, N_HID], F16)
    whr_sb = wpool.tile([N_HID, N_HID], F16)
    whu_sb = wpool.tile([N_HID, N_HID], F16)
    bu_sb = wpool.tile([N_HID, 1], F32)
    bh_sb = wpool.tile([N_HID, 1], F32)

    pa = papool.tile([N_HID, CHUNK], F32, name="pa")
    pb = [pbpool.tile([N_HID, HALF], F32, name=f"pb{k}") for k in range(2)]

    # weights go via the gpsimd DMA queue so the sync queue can issue the
    # first contents chunks immediately (the queues run in parallel and
    # each DMA costs ~0.9us of issue time on its queue)
    nc.gpsimd.dma_start(wu_sb[:], wu_d)
    nc.gpsimd.dma_start(bu_sb[:], bu_d)
    nc.gpsimd.dma_start(whl_sb[:], whl_d)
    nc.gpsimd.dma_start(whr_sb[:], whr_d)
    nc.gpsimd.dma_start(whu_sb[:], whu_d)
    nc.gpsimd.dma_start(bh_sb[:], bh_d)

    # embedding arenas, ping-pong between consecutive levels
    e_tiles = {}
    for j in range(DEPTH, 0, -1):
        tag = "ping" if j % 2 == 0 else "pong"
        e_tiles[j] = epool.tile([N_HID, PC_SIZES[j]], F16, tag=tag,
                                name=f"e{j}")

    # Station list, leaf level first.  Levels j <= TAIL_J are a pure
    # latency chain (sizes <= 1024), so their u-activations are hoisted
    # into one combined "uall" chunk: that removes the u-matmul+tanh from
    # every tail level's critical path.
    TAIL_J = 7
    TAIL_TOT = sum(PC_SIZES[j] for j in range(TAIL_J + 1))  # 2040
    stations = []
    for j in range(DEPTH, TAIL_J, -1):
        S = PC_SIZES[j]
        for a in range(0, S, CHUNK):
            stations.append(("leaf" if j == DEPTH else "inner", j, a,
                             min(CHUNK, S - a)))
    stations.append(("uall", TAIL_J, 0, TAIL_TOT))
    for j in range(TAIL_J, -1, -1):
        stations.append(("tailh", j, 0, PC_SIZES[j]))

    u_all = wpool.tile([N_HID, TAIL_TOT], F16, name="uall")
    state = {"leaf_ct": 0, "next_u": 0, "inner_ct": 0}
    u_tiles = {}

    def emit_u(i):
        kind, j, a, n = stations[i]
        col0 = PC_OFF[j] + a
        cst = cpool.tile([N_FEAT, CHUNK], F16, tag="cst", name=f"cst{i}")
        nc.sync.dma_start(cst[:, :n], c_d[:, col0:col0 + n])
        if kind == "leaf" and state["leaf_ct"] % 2 == 1:
            # odd leaf chunks go to the pb pair: two 1024-col halves
            state["leaf_ct"] += 1
            for k in range(2):
                h0 = k * HALF
                hn = min(HALF, max(0, n - h0))
                if hn == 0:
                    continue
                for s in range(0, hn, 512):
                    w = min(512, hn - s)
                    nc.tensor.matmul(pb[k][:, s:s + w], wu_sb[:],
                                     cst[:, h0 + s:h0 + s + w],
                                     start=True, stop=True)
                nc.scalar.activation(e_tiles[j][:, a + h0:a + h0 + hn],
                                     pb[k][:, :hn], TANH,
                                     bias=bu_sb[:, 0:1])
            return
        if kind == "leaf":
            state["leaf_ct"] += 1
        if kind == "inner" and state.get("first_inner", True):
            # at the leaf->inner boundary pa is still hot with the last
            # even leaf chunk; the pb pair is free one period earlier, so
            # the first inner u goes there to avoid a pipeline bubble
            state["first_inner"] = False
            u_sb = upool.tile([N_HID, CHUNK], F16, tag="u", name=f"u{i}")
            for k in range(2):
                h0 = k * HALF
                hn = min(HALF, max(0, n - h0))
                if hn == 0:
                    continue
                for s in range(0, hn, 512):
                    w = min(512, hn - s)
                    nc.tensor.matmul(pb[k][:, s:s + w], wu_sb[:],
                                     cst[:, h0 + s:h0 + s + w],
                                     start=True, stop=True)
                nc.scalar.activation(u_sb[:, h0:h0 + hn], pb[k][:, :hn],
                                     TANH, bias=bu_sb[:, 0:1])
            u_tiles[i] = u_sb
            return
        for s in range(0, n, 512):
            w = min(512, n - s)
            nc.tensor.matmul(pa[:, s:s + w], wu_sb[:], cst[:, s:s + w],
                             start=True, stop=True)
        if kind == "leaf":
            nc.scalar.activation(e_tiles[j][:, a:a + n], pa[:, :n], TANH,
                                 bias=bu_sb[:, 0:1])
        elif kind == "uall":
            nc.scalar.activation(u_all[:, :n], pa[:, :n], TANH,
                                 bias=bu_sb[:, 0:1])
        else:
            u_sb = upool.tile([N_HID, CHUNK], F16, tag="u", name=f"u{i}")
            nc.scalar.activation(u_sb[:, :n], pa[:, :n], TANH,
                                 bias=bu_sb[:, 0:1])
            u_tiles[i] = u_sb

    def pump_u(upto):
        # keep the u-stream two stations ahead of the h-stream
        while state["next_u"] < len(stations) and state["next_u"] <= upto:
            if stations[state["next_u"]][0] != "tailh":
                emit_u(state["next_u"])
            state["next_u"] += 1

    for i, (kind, j, a, n) in enumerate(stations):
        pump_u(i + 2)
        if kind in ("leaf", "uall"):
            continue
        S = PC_SIZES[j]
        eprev = e_tiles[j + 1]
        if kind == "inner":
            u_sb = u_tiles.pop(i)
        # process the h-stream in 1024-col halves, double-buffered across
        # pb0/pb1: matmuls filling one overlap the activation of the other
        for k, h0 in enumerate(range(0, n, HALF)):
            hn = min(HALF, n - h0)
            pt = pb[(k if kind == "inner" else j) % 2]
            # group by stationary weight to amortize LDWEIGHTS
            for s in range(0, hn, 512):
                w = min(512, hn - s)
                nc.tensor.matmul(pt[:, s:s + w], whl_sb[:],
                                 eprev[:, a + h0 + s:a + h0 + s + w],
                                 start=True, stop=False)
            for s in range(0, hn, 512):
                w = min(512, hn - s)
                nc.tensor.matmul(pt[:, s:s + w], whr_sb[:],
                                 eprev[:, S + a + h0 + s:S + a + h0 + s + w],
                                 start=False, stop=False)
            for s in range(0, hn, 512):
                w = min(512, hn - s)
                if kind == "inner":
                    uop = u_sb[:, h0 + s:h0 + s + w]
                else:
                    uo = PC_OFF[j] - PC_OFF[TAIL_J]
                    uop = u_all[:, uo + h0 + s:uo + h0 + s + w]
                nc.tensor.matmul(pt[:, s:s + w], whu_sb[:], uop,
                                 start=False, stop=True)
            if j > 0:
                nc.scalar.activation(e_tiles[j][:, a + h0:a + h0 + hn],
                                     pt[:, :hn], TANH, bias=bh_sb[:, 0:1])
            else:
                out_sb = opool.tile([N_HID, RPC], F32)
                nc.scalar.activation(out_sb[:], pt[:, 0:RPC], TANH,
                                     bias=bh_sb[:, 0:1])
                nc.sync.dma_start(out_d, out_sb[:])
        if kind == "inner":
            state["inner_ct"] += 1


def _preprocess(contents, children):
    """Relabel nodes so children of position p live at p, S+p; return
    per-core feature-major fp16 contents slices (leaf level first)."""
    contents = np.asarray(contents, dtype=np.float32)
    children = np.asarray(children)
    clipped = []
    for j in range(DEPTH):
        ch = children[INNER_OFF[j]:INNER_OFF[j + 1]]
        clipped.append(np.clip(ch, 0, LEVEL_SIZES[j + 1] - 1).astype(np.int64))

    per_core = []
    for d in range(N_CORES):
        # per-core walk: children of relabeled node p (of S) are at
        # positions p (left) and S + p (right) of the next level
        o = np.arange(d * RPC, (d + 1) * RPC, dtype=np.int64)
        segs = [contents[OFFSETS[0] + o]]
        for j in range(DEPTH):
            sel = clipped[j][o]
            o = np.concatenate([sel[:, 0], sel[:, 1]])
            segs.append(contents[OFFSETS[j + 1] + o])
        segs.reverse()  # leaf level first in the device buffer
        Cd = np.concatenate(segs, axis=0)  # [PC_TOTAL, 7]
        per_core.append(np.ascontiguousarray(Cd.T.astype(np.float16)))
    return per_core


def kernel(contents, children, w_u, b_u, w_h, b_h):
    contents = np.asarray(contents)
    children = np.asarray(children)
    w_u = np.asarray(w_u, dtype=np.float32)
    b_u = np.asarray(b_u, dtype=np.float32)
    w_h = np.asarray(w_h, dtype=np.float32)
    b_h = np.asarray(b_h, dtype=np.float32)

    per_core_c = _preprocess(contents, children)

    wu_t = np.ascontiguousarray(w_u.T.astype(np.float16))              # [7,128]
    whl_t = np.ascontiguousarray(w_h[:, 0:128].T.astype(np.float16))   # [128,128]
    whr_t = np.ascontiguousarray(w_h[:, 128:256].T.astype(np.float16))
    whu_t = np.ascontiguousarray(w_h[:, 256:384].T.astype(np.float16))
    bu_c = np.ascontiguousarray(b_u.reshape(N_HID, 1))
    bh_c = np.ascontiguousarray(b_h.reshape(N_HID, 1))

    if "nc" not in _COMPILED:
        _COMPILED["nc"] = _build_program()
    nc = _COMPILED["nc"]

    in_maps = []
    for d in range(N_CORES):
        in_maps.append({
            "c": per_core_c[d],
            "wu": wu_t, "whl": whl_t, "whr": whr_t, "whu": whu_t,
            "bu": bu_c, "bh": bh_c,
        })
    res = run_bass_kernel_spmd(nc, in_maps, list(range(N_CORES)))

    out = np.empty((B, N_HID), dtype=np.float32)
    for d in range(N_CORES):
        out[d * RPC:(d + 1) * RPC, :] = res.results[d]["out"].T
    return out
